# revision 7
# baseline (speedup 1.0000x reference)
"""Trainium2 Bass kernel for the 32-iteration 3x3 survival automaton.

Problem: x is a 4096x4096 binary fp32 grid. 32 iterations of:
    keep cell iff its 8-neighbor live count > 3  (zero 'SAME' padding)
Output: scalar sum(x) - sum(y_final).

v2 strategy (vs the bf16 2-matmul-pass baseline): fp8 DoubleRow matmuls
fuse the two vertical-band passes into ~1.4 passes per element.

  - Row-shard: core c owns rows [512c, 512c+512) plus a 32-row halo per
    side, consumed one row per iteration (zero inter-core traffic).
  - State per tile: [r, 3, 4144] fp8e4 planes: plane0 = B (precombined
    left+right columns, A-columns only; zero over Bbar columns),
    plane1 = y, plane2 = y<<2 (DMA-maintained, Bbar columns only).
  - A-columns (first 512*NA): ONE DoubleRow matmul per 512 cols:
        psum = Tri@B + (Tri+4I)@y          (pair = (plane0, plane1))
    B comes from a DVE fp8 add. Bbar columns: TWO DoubleRow matmuls:
        psum = (Tri|Tri4I)@(0, y) + (Tri|Tri)@(y<<1, y>>1)
    where y>>1 is plane2[c-1]; plane0=0 lets the center-term MM share
    the A-columns' stationary, so only 2 stationaries cycle per tile.
  - psum = n + 5y exactly (n = live 8-neighbor count); survive <=> s>8.5.
    Threshold from PSUM in 2048-col units: ScalarE Sigmoid(120*(s-8.5))
    (saturates to exact 1.0/0.0) and VectorE is_gt, split for balance.
  - Seams: five 128-row tiles at stride 120 (8-row overlap); outer rows
    go stale 1 row/iter and are refreshed every KSH=4 iters by tiny DMAs
    (y plane only -- B and plane2 are rebuilt from y every iteration).
  - Final reduction: accum_out on the last iteration's thresholds, then
    masked ones-vector matmuls; host sums 8 partials.
"""

import sys

if '/opt/trn_rl_repo' not in sys.path:
    sys.path.insert(0, '/opt/trn_rl_repo')

from contextlib import ExitStack, contextmanager

import ml_dtypes
import numpy as np

import concourse.bass as bass
import concourse.tile as tile
from concourse import bacc, mybir
from concourse.bass_utils import run_bass_kernel_spmd

# ---------------------------------------------------------------- geometry
H = W = 4096
NCORES = 8
OWN = H // NCORES            # 512 rows owned per core
HALO = 32                    # rows of redundant compute per side
SLAB_R = OWN + 2 * HALO + 2  # 578 (incl. 1 guard row each side)
CW = 4144                    # plane width (16-aligned); data cols [16, 4112)
C0 = 16                      # first data col
NT = 5                       # SBUF row-tiles per slab
KSH = 4                      # seam shrink depth: refresh every KSH iters
STRIDE = 128 - 2 * KSH       # 120 (8-row overlap between tiles)
OFF = [t * STRIDE for t in range(NT)]              # 0,120,240,360,480
RT = [min(128, SLAB_R - o) for o in OFF]           # 128,128,128,128,98
MMW = 512                    # matmul output free size (1 PSUM bank)
NU8 = W // MMW               # 8 matmul units per row-tile
NA = 4                       # A-mode units (DVE-built B); rest are Bbar
CB = C0 + NA * MMW           # 2064: first Bbar col
THW = 1024                   # threshold granularity: 2 PSUM banks
NTH = W // THW               # 2 threshold units per row-tile

# threshold split: per tile, units handled by ScalarE (sigmoid); the rest
# go to VectorE (exact is_gt). Tuned for engine balance.
ACT_UNITS = [3, 3, 3, 3, 3]

F32 = mybir.dt.float32
F8 = mybir.dt.float8e4
DR = mybir.MatmulPerfMode.DoubleRow


@contextmanager
def _no_ldweights():
    """Emit InstMatmult with ldweights=False: reuse the PE array's currently
    loaded stationary instead of reloading per matmul."""
    orig = mybir.InstMatmult

    def mk(*a, **kw):
        kw['ldweights'] = False
        return orig(*a, **kw)

    mybir.InstMatmult = mk
    try:
        yield
    finally:
        mybir.InstMatmult = orig


def _ldw_sig(inst):
    """Signature of the stationary operand an InstLdweights loads."""
    ap = inst.ins[0]
    return (getattr(ap, 'memref', None), getattr(ap, 'offset', None),
            str(getattr(ap, 'ap', None)), str(inst.tile_position),
            str(inst.tile_size), str(getattr(inst, 'perf_mode', None)),
            str(getattr(inst, 'is_transpose', None)))


def _dedup_ldweights(nc):
    """Remove InstLdweights that reload the stationary already in the PE
    array (same weights AP, only non-loading Matmults in between). Waits on
    a removed load are pushed onto the next PE instruction; loads carrying
    semaphore updates are kept."""
    removed = 0
    for f in nc.m.functions:
        for blk in f.blocks:
            cur = None
            out = []
            pending_waits = []
            for inst in blk.instructions:
                if isinstance(inst, mybir.InstLdweights):
                    sig = _ldw_sig(inst)
                    si = inst.sync_info
                    has_upd = si is not None and len(si.on_update) > 0
                    if sig == cur and not has_upd:
                        if si is not None and len(si.on_wait) > 0:
                            pending_waits.extend(si.on_wait)
                        removed += 1
                        continue
                    cur = sig
                elif isinstance(inst, mybir.InstMatmult):
                    if inst.is_transpose or getattr(inst, 'ldweights', None) is not False:
                        cur = None
                elif type(inst).__name__ == 'InstMatmultMx':
                    cur = None
                if pending_waits and isinstance(
                        inst, (mybir.InstLdweights, mybir.InstMatmult)):
                    si = inst.sync_info
                    if si is None:
                        inst.sync_info = mybir.SyncInfo(
                            on_wait=list(pending_waits), on_update=[])
                    else:
                        si.on_wait = list(si.on_wait) + pending_waits
                    pending_waits = []
                out.append(inst)
            assert not pending_waits
            if len(out) != len(blk.instructions):
                blk.instructions[:] = out
    return removed


def _build(iters: int):
    nc = bacc.Bacc("TRN2", target_bir_lowering=False, debug=False)
    x_d = nc.dram_tensor("x", [SLAB_R, 3, CW], F8, kind="ExternalInput").ap()
    sa_d = nc.dram_tensor("tri", [128, 2, 128], F8, kind="ExternalInput").ap()
    sb_d = nc.dram_tensor("m16", [128, 2, 128], F8, kind="ExternalInput").ap()
    rmask_d = nc.dram_tensor("rmask", [NT, 128], F32, kind="ExternalInput").ap()
    out_d = nc.dram_tensor("ysum", [1, 1], F32, kind="ExternalOutput").ap()

    add = mybir.AluOpType.add

    with tile.TileContext(nc) as tc, ExitStack() as ctx:
        const_pool = ctx.enter_context(tc.tile_pool(name="const", bufs=1))
        ypool = ctx.enter_context(tc.tile_pool(name="y", bufs=1))

        sa_sb = const_pool.tile([128, 2, 128], F8, tag="sa")
        nc.sync.dma_start(sa_sb[:], sa_d[:])
        sb_sb = const_pool.tile([128, 2, 128], F8, tag="sb")
        nc.sync.dma_start(sb_sb[:], sb_d[:])
        rmask_sb = []
        for t in range(NT):
            rm = const_pool.tile([128, 1], F32, tag=f"rmask{t}", name=f"rmask{t}")
            nc.sync.dma_start(rm[:], rmask_d[t:t + 1, :])
            rmask_sb.append(rm)
        bias_sb = const_pool.tile([128, 1], F32, tag="biasc", name="biasc")
        nc.gpsimd.memset(bias_sb[:], -1020.0)

        y_sb = [ypool.tile([RT[t], 3, CW], F8, tag=f"y{t}", name=f"y{t}")
                for t in range(NT)]

        # load (host already built the 3 fp8 planes)
        for t in range(NT):
            nc.sync.dma_start(y_sb[t][:], x_d[OFF[t]:OFF[t] + RT[t], :, :])

        def emit_b(t):
            # B = y<<1 + y>>1 over A columns (DVE), y<<2 over Bbar (DMA)
            r = RT[t]
            half = (CB - C0) // 2
            for h in range(2):
                a, b = C0 + h * half, C0 + (h + 1) * half
                nc.vector.tensor_tensor(
                    y_sb[t][0:r, 0, a:b],
                    y_sb[t][0:r, 1, a - 1:b - 1],
                    y_sb[t][0:r, 1, a + 1:b + 1], op=add)
            nc.scalar.dma_start(
                y_sb[t][0:r, 2, CB - 1:C0 + W],
                y_sb[t][0:r, 1, CB + 1:C0 + W + 2])

        def emit_seam(t):
            # refresh the 2*KSH-row overlap between tiles t and t+1 (each
            # tile's outer KSH rows go stale over KSH iterations); y plane
            # only -- B/plane2 are rebuilt from y each iteration.
            nc.sync.dma_start(y_sb[t][128 - KSH:128, 1, :],
                              y_sb[t + 1][KSH:2 * KSH, 1, :])
            nc.sync.dma_start(y_sb[t + 1][0:KSH, 1, :],
                              y_sb[t][STRIDE:STRIDE + KSH, 1, :])

        acc_sb = [[const_pool.tile([128, 1], F32, tag=f"acc{t}_{u}",
                                   name=f"acc{t}_{u}") for u in range(NTH)]
                  for t in range(NT)]

        def emit_mms_thresholds(psum_pool, it, t, accum=False):
            r = RT[t]
            psums = [psum_pool.tile([r, THW], F32, tag="ps",
                                    name=f"ps_{it}_{t}_{v}")
                     for v in range(NTH)]
            # group 1: stationary (Tri | Tri+4I), pair (plane0, plane1).
            # A-units finish here (plane0 = B); Bbar units (plane0 = 0)
            # accumulate the side-column group-2 MM on top.
            first = True
            for u in range(NU8):
                c0 = C0 + u * MMW
                args = (psums[u // 2][:, (u % 2) * MMW:(u % 2 + 1) * MMW],
                        sa_sb[0:r, :, 0:r],
                        y_sb[t][0:r, 0:2, c0:c0 + MMW])
                kw = dict(start=True, stop=(u < NA), perf_mode=DR)
                if first:
                    nc.tensor.matmul(*args, **kw)
                    first = False
                else:
                    with _no_ldweights():
                        nc.tensor.matmul(*args, **kw)
            # group 2: stationary (Tri | Tri), pair (y<<1, y>>1)
            first = True
            for u in range(NA, NU8):
                c0 = C0 + u * MMW
                args = (psums[u // 2][:, (u % 2) * MMW:(u % 2 + 1) * MMW],
                        sb_sb[0:r, :, 0:r],
                        y_sb[t][0:r, 1:3, c0 - 1:c0 + MMW - 1])
                kw = dict(start=False, stop=True, perf_mode=DR)
                if first:
                    nc.tensor.matmul(*args, **kw)
                    first = False
                else:
                    with _no_ldweights():
                        nc.tensor.matmul(*args, **kw)
            for v in range(NTH):
                dst = y_sb[t][0:r, 1, C0 + v * THW:C0 + (v + 1) * THW]
                aout = acc_sb[t][v][0:r, 0:1] if accum else None
                if v != 0:
                    nc.scalar.activation(
                        dst, psums[v][:],
                        mybir.ActivationFunctionType.Sigmoid,
                        bias=bias_sb[0:r, 0:1], scale=120.0,
                        accum_out=aout)
                else:
                    if accum:
                        nc.vector.tensor_scalar(
                            dst, psums[v][:], 8.5, 0.0,
                            op0=mybir.AluOpType.is_gt,
                            op1=mybir.AluOpType.add, accum_out=aout)
                    else:
                        nc.vector.tensor_scalar(
                            dst, psums[v][:], 8.5, None,
                            op0=mybir.AluOpType.is_gt)

        # Software-pipelined wavefront with KSH-iter seam shrinkage, same
        # cadence as the bf16 baseline: on non-refresh boundaries a tile's
        # next-iteration B-pass is emitted right after its threshold so
        # TensorE rolls across the iteration boundary with no bubble; on
        # refresh boundaries seams go first once both neighbors thresholded.
        with tc.tile_pool(name="ps", bufs=4, space="PSUM") as psum_pool:
            for it in range(iters):
                last = it == iters - 1
                refresh = (it % KSH == KSH - 1) and not last
                for t in range(NT):
                    emit_mms_thresholds(psum_pool, it, t, accum=last)
                    if last:
                        continue
                    if refresh:
                        if t >= 1:
                            emit_seam(t - 1)
                        if t >= 2:
                            emit_b(t - 2)
                    else:
                        emit_b(t)
                if not last and refresh:
                    emit_b(NT - 2)
                    emit_b(NT - 1)

        # masked dot of the per-row accumulators from the last iteration's
        # thresholds: ysum = sum_t rmask[t] . (row sums of tile t)
        with tc.tile_pool(name="sps", bufs=1, space="PSUM") as spsum_pool:
            sps = spsum_pool.tile([1, 1], F32, tag="sum", name="sps")
            n_mm = NT * NTH
            k = 0
            for t in range(NT):
                for u in range(NTH):
                    nc.tensor.matmul(
                        sps[:], rmask_sb[t][0:RT[t], 0:1],
                        acc_sb[t][u][0:RT[t], 0:1],
                        start=(k == 0), stop=(k == n_mm - 1))
                    k += 1
            ssb = const_pool.tile([1, 1], F32, tag="ssum", name="ssb")
            nc.vector.tensor_copy(ssb[:], sps[:])
            nc.sync.dma_start(out_d[:], ssb[:])

    _dedup_ldweights(nc)
    # After dedup, the "most recent ldweights" a matmul's extra waits would
    # be moved to can sit many matmuls earlier in the PE stream — waiting
    # there can deadlock against producers scheduled in between. Skip the
    # pass; generate_event_semaphores enforces the 1-wait constraint by
    # splitting waits into standalone event-sem instructions in place.
    nc.move_matmul_waits_to_ldweights = lambda: None
    nc.compile()
    return nc


def _consts():
    i = np.arange(128)
    tri = (np.abs(i[:, None] - i[None, :]) <= 1).astype(np.float32)
    tri4 = tri + 4.0 * np.eye(128, dtype=np.float32)
    sa = np.stack([tri, tri4], axis=1)   # [K, 2, M]
    sb1 = np.stack([tri, tri], axis=1)
    # valid-row masks for the final sum: slab rows [33, 545) are the owned
    # 512 rows; each row is summed from the tile where it is seam-valid
    # (interior partitions after the last iteration).
    rmask = np.zeros((NT, 128), np.float32)
    bounds = [(33, 124), (4, 124), (4, 124), (4, 124), (4, 65)]
    for t, (a, b) in enumerate(bounds):
        rmask[t, a:b] = 1.0
    assert sum(b - a for a, b in bounds) == OWN
    f8 = ml_dtypes.float8_e4m3
    return sa.astype(f8), sb1.astype(f8), rmask


def _slabs(x: np.ndarray):
    f8 = ml_dtypes.float8_e4m3
    gr = H + 2 * HALO + 2    # 4162 padded rows
    yg = np.zeros((gr, CW), np.float32)
    yg[HALO + 1:HALO + 1 + H, C0:C0 + W] = x   # 0/1: exact in fp8
    bg = np.zeros((gr, CW), np.float32)
    bg[:, C0:CB] = yg[:, C0 - 1:CB - 1] + yg[:, C0 + 1:CB + 1]
    p2 = np.zeros((gr, CW), np.float32)
    p2[:, CB - 1:C0 + W] = yg[:, CB + 1:C0 + W + 2]
    g = np.stack([bg, yg, p2], axis=1).astype(f8)   # [gr, 3, CW]
    return [np.ascontiguousarray(g[c * OWN:c * OWN + SLAB_R])
            for c in range(NCORES)]


_CACHE = {}


def _get_nc(iters: int):
    if iters not in _CACHE:
        _CACHE[iters] = _build(iters)
    return _CACHE[iters]


def kernel(x: np.ndarray, convs) -> np.ndarray:
    iters = int(convs)
    x = np.asarray(x, np.float32)
    assert x.shape == (H, W)
    nc = _get_nc(iters)
    sa, sb1, rmask = _consts()
    in_maps = [{"x": s, "tri": sa, "m16": sb1, "rmask": rmask}
               for s in _slabs(x)]
    res = run_bass_kernel_spmd(nc, in_maps, core_ids=list(range(NCORES)))
    y_sum = sum(float(res.results[c]["ysum"][0, 0]) for c in range(NCORES))
    x_sum = float(x.astype(np.float64).sum())
    return np.float32(x_sum - y_sum)


if __name__ == "__main__":
    rng = np.random.default_rng(0)
    x = np.round(rng.random((H, W))).astype(np.float32)
    got = kernel(x, 32)
    from scipy import signal
    K = np.array([[1, 1, 1], [1, 0, 1], [1, 1, 1]], np.float32)
    y = x.copy()
    for _ in range(32):
        s = signal.convolve2d(y, K, mode='same')
        y = np.where(s > 3.0, y, 0).astype(np.float32)
    want = x.sum(dtype=np.float64) - y.sum(dtype=np.float64)
    print(f"got {got}, want {want}, rel {abs(got - want) / abs(want):.3e}")
